# revision 23
# baseline (speedup 1.0000x reference)
"""ConvMod3d (StyleGAN-style modulated 3x3x3 conv, N=4 groups) on 8 trn2 cores.

Sharding: 8 shards = 4 samples x 2 depth-halves. Each core convolves a
25-plane input slab (64ch x 48x48) against its sample's modulated 64x64x27
weights, producing 23 output planes. Style modulation/demodulation of the
tiny weight tensor happens on host; the conv (99.8% of FLOPs) on device.

Per output plane d': 27 taps, each a [Cin=64 -> Cout=64] matmul over the
flattened 48x48 plane with a shifted read offset; invalid edge columns
(w'>=46, h'>=46) are computed and discarded on the host side.

PE packing (trn2 constraints: row tiling crashes the device; alternating
contract sizes back-to-back costs 2.2x, so same-contract matmuls are kept
contiguous). Taps are packed two-per-matmul on the contraction dim via
stacked SBUF windows:
- W[p]  = plane p (partitions 0-63) | plane p+1 (64-127): fuses the
  (kd=0,kd=1) tap pairs -> 9 contract-128 streams per output plane.
- W2[p] = plane p | plane p shifted +48 cols (one h row): fuses the
  (kd=2, kh=0/1) pairs -> 3 contract-128 streams; the 3 (kd=2,kh=2)
  taps stay contract-64 on W2's lower half. (A third +1-col-shift
  family packing those too was tried and lost: its extra HBM traffic
  stalls the PE on window waits.)
Two output planes run concurrently on PE col strips (plane A accumulates
in one PSUM bank partitions 0-63, plane B in another, partitions 64-127).
Matmuls in bf16 (fp32 PSUM accumulation).
"""

import time

import numpy as np
import ml_dtypes

import concourse.bacc as bacc
import concourse.bass as bass
import concourse.tile as tile
from concourse import mybir
from concourse.bass_utils import run_bass_kernel_spmd

EPS = 1e-8
N, CIN, COUT = 4, 64, 64
DHW, K = 48, 3
DOUT = DHW - K + 1          # 46
HALF = DOUT // 2            # 23 output planes per core
P_IN = HALF + K - 1         # 25 input planes per core
PLANE = DHW * DHW           # 2304
PAD_COLS = 192              # tail slack so shifted reads stay in-bounds
XS_COLS = P_IN * PLANE + PAD_COLS
WCOLS = PLANE + PAD_COLS - 64   # window columns (2432); max offset used 98+2207
PLANE_OUT = (DHW - 2) * DHW     # 2208 computed output cols (h' rows 0-45)
NTAPS = K * K * K           # 27
GROUP = 2                   # output planes per group (PSUM col strips)
NGROUPS = (HALF + GROUP - 1) // GROUP
CHUNKS = [(0, 512), (512, 512), (1024, 512), (1536, 512), (2048, 160)]
NCORES = 8
NWBLK = 15                  # weight blocks of 64 cols

F32 = mybir.dt.float32
MM_DT = mybir.dt.bfloat16
NP_MM = np.dtype(ml_dtypes.bfloat16)

_CACHE = {}
LAST_RESULTS = None  # BassKernelResults of the most recent device run


def _build_bass():
    nc = bacc.Bacc()
    xs = nc.declare_dram_parameter("xs", [CIN, XS_COLS], MM_DT, isOutput=False)
    wt = nc.declare_dram_parameter("wt", [128, NWBLK * COUT], MM_DT, isOutput=False)
    bt = nc.declare_dram_parameter("bt", [128, 1], F32, isOutput=False)
    y = nc.declare_dram_parameter(
        "y", [NGROUPS, GROUP * 64, PLANE_OUT], F32, isOutput=True)

    with tile.TileContext(nc) as tc:
        with (
            tc.tile_pool(name="const", bufs=1) as cpool,
            tc.tile_pool(name="xpool", bufs=20) as xpool,
            tc.tile_pool(name="opool", bufs=3) as opool,
            tc.tile_pool(name="ppool", bufs=8, space="PSUM") as ppool,
        ):
            wtile = cpool.tile([128, NWBLK * COUT], MM_DT)
            nc.sync.dma_start(out=wtile[:, :], in_=wt[:, :])
            btile = cpool.tile([128, 1], F32)
            nc.sync.dma_start(out=btile[:, :], in_=bt[:, :])

            windows = {}

            UPSHIFT = {"w": PLANE, "w2": DHW}

            def load_window(fam, p, split=False):
                # upper half holds the lower plane shifted by UPSHIFT[fam].
                # split=True loads in column halves so early matmuls (which
                # only touch low columns) start before the full window lands.
                key = (fam, p)
                if key in windows or p >= P_IN:
                    return
                xw = xpool.tile([128, WCOLS], MM_DT, tag="xw", name="xw")
                base = p * PLANE
                up = base + UPSHIFT[fam]
                cuts = [0, 1280, WCOLS] if split else [0, WCOLS]
                for a, b in zip(cuts, cuts[1:]):
                    nc.sync.dma_start(out=xw[0:64, a:b],
                                      in_=xs[:, base + a:base + b])
                    if up + WCOLS <= XS_COLS:
                        nc.sync.dma_start(out=xw[64:128, a:b],
                                          in_=xs[:, up + a:up + b])
                windows[key] = xw

            def ensure_group_windows(g, split=False):
                if g >= NGROUPS:
                    return
                for d in range(g * GROUP, min(HALF, (g + 1) * GROUP)):
                    load_window("w", d, split=split)
                    load_window("w2", d + 2, split=split)

            ensure_group_windows(0, split=True)
            for g0 in range(1, 4):
                ensure_group_windows(g0)

            for grp in range(NGROUPS):
                dps = [d for d in range(grp * GROUP, (grp + 1) * GROUP)
                       if d < HALF]
                ensure_group_windows(grp + 4)
                nparts = 64 * len(dps)

                ot = opool.tile([128, PLANE_OUT], F32, tag="ot")
                for cidx, (c0, csz) in enumerate(CHUNKS):
                    pss = [ppool.tile([128, 512], F32, tag="ps", name="ps")
                           for _ in dps]
                    # (j, ci): j 0-8 fused kd01 (c128, W[dp], off
                    # kh*48+kw); j 9-11 fused kd2 kh01 (c128, W2[dp+2],
                    # off kw); j 12-14 kd2 kh2 (c64, W2[dp+2] lower,
                    # off 96+kw). Same-contract matmuls contiguous;
                    # serpentine the kind order across chunks so chunk
                    # boundaries don't add a contract-size switch.
                    # (Tap-outer over multiple chunks was tried to
                    # amortize LDWEIGHTS and lost: per-matmul PSUM bank
                    # cycling costs more than the weight reloads.)
                    jorder = list(range(NWBLK))
                    if cidx % 2 == 1:
                        jorder = jorder[12:] + jorder[:12]
                    mms = [(j, ci) for j in jorder
                           for ci in range(len(dps))]
                    first_ci = {}
                    last_ci = {}
                    for idx, (j, ci) in enumerate(mms):
                        first_ci.setdefault(ci, idx)
                        last_ci[ci] = idx
                    for idx, (j, ci) in enumerate(mms):
                        dst = pss[ci][ci * 64:(ci + 1) * 64, 0:csz]
                        if j < 9:
                            kh, kw = divmod(j, 3)
                            win = windows[("w", dps[ci])]
                            off = kh * DHW + kw + c0
                            rows = 128
                        elif j < 12:
                            kw = j - 9
                            win = windows[("w2", dps[ci] + 2)]
                            off = kw + c0
                            rows = 128
                        else:
                            kw = j - 12
                            win = windows[("w2", dps[ci] + 2)]
                            off = 2 * DHW + kw + c0
                            rows = 64
                        nc.tensor.matmul(
                            dst,
                            wtile[0:rows, j * 64:(j + 1) * 64],
                            win[0:rows, off:off + csz],
                            start=(idx == first_ci[ci]),
                            stop=(idx == last_ci[ci]),
                        )
                    for ci in range(len(dps)):
                        nc.scalar.activation(
                            ot[ci * 64:(ci + 1) * 64, c0:c0 + csz],
                            pss[ci][ci * 64:(ci + 1) * 64, 0:csz],
                            mybir.ActivationFunctionType.Identity,
                            bias=btile[ci * 64:(ci + 1) * 64, :],
                        )
                nc.scalar.dma_start(out=y[grp, 0:nparts, :], in_=ot[0:nparts, :])
    nc.compile()
    return nc


def _prep_in_maps(x, s, style_weight, style_bias, weight, bias):
    style = s @ style_weight.T + style_bias                      # [N, Cin]
    wm = weight[None] * style[:, None, :, None, None, None]      # [N,Co,Ci,k,k,k]
    wm = wm * (1.0 / np.sqrt((wm * wm).sum(axis=(2, 3, 4, 5), keepdims=True) + EPS))
    wk = wm.transpose(0, 2, 3, 4, 5, 1)                          # [N,Ci,kd,kh,kw,Co]
    wfull = np.zeros((N, 128, NWBLK * COUT), np.float32)
    for j in range(9):
        kh, kw = divmod(j, 3)
        wfull[:, 0:64, j * 64:(j + 1) * 64] = wk[:, :, 0, kh, kw, :]
        wfull[:, 64:128, j * 64:(j + 1) * 64] = wk[:, :, 1, kh, kw, :]
    for kw in range(3):
        j = 9 + kw
        wfull[:, 0:64, j * 64:(j + 1) * 64] = wk[:, :, 2, 0, kw, :]
        wfull[:, 64:128, j * 64:(j + 1) * 64] = wk[:, :, 2, 1, kw, :]
    for kw in range(3):
        j = 12 + kw
        wfull[:, 0:64, j * 64:(j + 1) * 64] = wk[:, :, 2, 2, kw, :]
    wfull = np.ascontiguousarray(wfull.astype(NP_MM))
    bt = np.ascontiguousarray(
        np.tile(bias[:, None], (2, 1)), dtype=np.float32)        # [128,1]

    in_maps = []
    for core in range(NCORES):
        n, h = divmod(core, 2)
        d0 = h * HALF
        xsl = x[n, :, d0:d0 + P_IN].reshape(CIN, P_IN * PLANE)
        xsl = np.concatenate(
            [xsl, np.zeros((CIN, PAD_COLS), np.float32)], axis=1)
        in_maps.append({
            "xs": np.ascontiguousarray(xsl.astype(NP_MM)),
            "wt": wfull[n],
            "bt": bt,
        })
    return in_maps


def _gather(results):
    y = np.empty((N, COUT, DOUT, DOUT, DOUT), np.float32)
    for core in range(NCORES):
        n, h = divmod(core, 2)
        planes = results[core]["y"].reshape(
            NGROUPS * GROUP, COUT, DHW - 2, DHW)[:HALF]
        y[n, :, h * HALF:(h + 1) * HALF] = (
            planes[:, :, :, :DOUT].transpose(1, 0, 2, 3))
    return y


def kernel(x, s, style_weight, style_bias, weight, bias):
    global LAST_RESULTS
    x = np.asarray(x, np.float32)
    s = np.asarray(s, np.float32)
    style_weight = np.asarray(style_weight, np.float32)
    style_bias = np.asarray(style_bias, np.float32)
    weight = np.asarray(weight, np.float32)
    bias = np.asarray(bias, np.float32)

    if "nc" not in _CACHE:
        _CACHE["nc"] = _build_bass()
    in_maps = _prep_in_maps(x, s, style_weight, style_bias, weight, bias)
    res = None
    for attempt in range(3):
        try:
            res = run_bass_kernel_spmd(_CACHE["nc"], in_maps, list(range(NCORES)))
            break
        except Exception:
            if attempt == 2:
                raise
            time.sleep(30)  # transient device wedge; recovers on its own
    LAST_RESULTS = res
    return _gather(res.results)


# revision 24
# speedup vs baseline: 1.0512x; 1.0512x over previous
"""ConvMod3d (StyleGAN-style modulated 3x3x3 conv, N=4 groups) on 8 trn2 cores.

Sharding: 8 shards = 4 samples x 2 depth-halves. Each core convolves a
25-plane input slab (64ch x 48x48) against its sample's modulated 64x64x27
weights, producing 23 output planes. Style modulation/demodulation of the
tiny weight tensor happens on host; the conv (99.8% of FLOPs) on device.

Per output plane d': 27 taps, each a [Cin=64 -> Cout=64] matmul over the
flattened 48x48 plane with a shifted read offset; invalid edge columns
(w'>=46, h'>=46) are computed and discarded on the host side.

PE packing (trn2 constraints: row tiling crashes the device; alternating
contract sizes back-to-back costs 2.2x, so same-contract matmuls are kept
contiguous). Taps are packed two-per-matmul on the contraction dim via
stacked SBUF windows:
- W[p]  = plane p (partitions 0-63) | plane p+1 (64-127): fuses the
  (kd=0,kd=1) tap pairs -> 9 contract-128 streams per output plane.
- W2[p] = plane p | plane p shifted +48 cols (one h row): fuses the
  (kd=2, kh=0/1) pairs -> 3 contract-128 streams; the 3 (kd=2,kh=2)
  taps stay contract-64 on W2's lower half. (A third +1-col-shift
  family packing those too was tried and lost: its extra HBM traffic
  stalls the PE on window waits.)
Two output planes run concurrently on PE col strips (plane A accumulates
in one PSUM bank partitions 0-63, plane B in another, partitions 64-127).
Matmuls in bf16 (fp32 PSUM accumulation).
"""

import time

import numpy as np
import ml_dtypes

import concourse.bacc as bacc
import concourse.bass as bass
import concourse.tile as tile
from concourse import mybir
from concourse.bass_utils import run_bass_kernel_spmd

EPS = 1e-8
N, CIN, COUT = 4, 64, 64
DHW, K = 48, 3
DOUT = DHW - K + 1          # 46
HALF = DOUT // 2            # 23 output planes per core
P_IN = HALF + K - 1         # 25 input planes per core
PLANE = DHW * DHW           # 2304
PAD_COLS = 192              # tail slack so shifted reads stay in-bounds
XS_COLS = P_IN * PLANE + PAD_COLS
WCOLS = PLANE + PAD_COLS - 64   # window columns (2432); max offset used 98+2207
PLANE_OUT = (DHW - 2) * DHW     # 2208 computed output cols (h' rows 0-45)
NTAPS = K * K * K           # 27
GROUP = 2                   # output planes per group (PSUM col strips)
NGROUPS = (HALF + GROUP - 1) // GROUP
CHUNKS = [(0, 512), (512, 512), (1024, 512), (1536, 512), (2048, 160)]
NCORES = 8
NWBLK = 15                  # weight blocks of 64 cols

F32 = mybir.dt.float32
MM_DT = mybir.dt.bfloat16
NP_MM = np.dtype(ml_dtypes.bfloat16)

_CACHE = {}
LAST_RESULTS = None  # BassKernelResults of the most recent device run


def _build_bass():
    nc = bacc.Bacc()
    xs = nc.declare_dram_parameter("xs", [CIN, XS_COLS], MM_DT, isOutput=False)
    wt = nc.declare_dram_parameter("wt", [128, NWBLK * COUT], MM_DT, isOutput=False)
    bt = nc.declare_dram_parameter("bt", [128, 1], F32, isOutput=False)
    y = nc.declare_dram_parameter(
        "y", [NGROUPS, GROUP * 64, PLANE_OUT], F32, isOutput=True)

    with tile.TileContext(nc) as tc:
        with (
            tc.tile_pool(name="const", bufs=1) as cpool,
            tc.tile_pool(name="xpool", bufs=16) as xpool,
            tc.tile_pool(name="opool", bufs=3) as opool,
            tc.tile_pool(name="ppool", bufs=8, space="PSUM") as ppool,
        ):
            wtile = cpool.tile([128, NWBLK * COUT], MM_DT)
            nc.sync.dma_start(out=wtile[:, :], in_=wt[:, :])
            btile = cpool.tile([128, 1], F32)
            nc.sync.dma_start(out=btile[:, :], in_=bt[:, :])

            windows = {}

            UPSHIFT = {"w": PLANE, "w2": DHW}

            def load_window(fam, p, split=False):
                # upper half holds the lower plane shifted by UPSHIFT[fam].
                # split=True loads in column halves so early matmuls (which
                # only touch low columns) start before the full window lands.
                key = (fam, p)
                if key in windows or p >= P_IN:
                    return
                xw = xpool.tile([128, WCOLS], MM_DT, tag="xw", name="xw")
                base = p * PLANE
                up = base + UPSHIFT[fam]
                cuts = [0, 1280, WCOLS] if split else [0, WCOLS]
                for a, b in zip(cuts, cuts[1:]):
                    nc.sync.dma_start(out=xw[0:64, a:b],
                                      in_=xs[:, base + a:base + b])
                    if up + WCOLS <= XS_COLS:
                        nc.sync.dma_start(out=xw[64:128, a:b],
                                          in_=xs[:, up + a:up + b])
                windows[key] = xw

            def ensure_group_windows(g, split=False):
                if g >= NGROUPS:
                    return
                for d in range(g * GROUP, min(HALF, (g + 1) * GROUP)):
                    load_window("w", d, split=split)
                    load_window("w2", d + 2, split=split)

            ensure_group_windows(0, split=True)
            for g0 in range(1, 3):
                ensure_group_windows(g0)

            for grp in range(NGROUPS):
                dps = [d for d in range(grp * GROUP, (grp + 1) * GROUP)
                       if d < HALF]
                ensure_group_windows(grp + 3)
                nparts = 64 * len(dps)

                ot = opool.tile([128, PLANE_OUT], F32, tag="ot")
                for cidx, (c0, csz) in enumerate(CHUNKS):
                    pss = [ppool.tile([128, 512], F32, tag="ps", name="ps")
                           for _ in dps]
                    # (j, ci): j 0-8 fused kd01 (c128, W[dp], off
                    # kh*48+kw); j 9-11 fused kd2 kh01 (c128, W2[dp+2],
                    # off kw); j 12-14 kd2 kh2 (c64, W2[dp+2] lower,
                    # off 96+kw). Same-contract matmuls contiguous;
                    # serpentine the kind order across chunks so chunk
                    # boundaries don't add a contract-size switch.
                    # (Tap-outer over multiple chunks was tried to
                    # amortize LDWEIGHTS and lost: per-matmul PSUM bank
                    # cycling costs more than the weight reloads.)
                    jorder = list(range(NWBLK))
                    if cidx % 2 == 1:
                        jorder = jorder[12:] + jorder[:12]
                    mms = [(j, ci) for j in jorder
                           for ci in range(len(dps))]
                    first_ci = {}
                    last_ci = {}
                    for idx, (j, ci) in enumerate(mms):
                        first_ci.setdefault(ci, idx)
                        last_ci[ci] = idx
                    for idx, (j, ci) in enumerate(mms):
                        dst = pss[ci][ci * 64:(ci + 1) * 64, 0:csz]
                        if j < 9:
                            kh, kw = divmod(j, 3)
                            win = windows[("w", dps[ci])]
                            off = kh * DHW + kw + c0
                            rows = 128
                        elif j < 12:
                            kw = j - 9
                            win = windows[("w2", dps[ci] + 2)]
                            off = kw + c0
                            rows = 128
                        else:
                            kw = j - 12
                            win = windows[("w2", dps[ci] + 2)]
                            off = 2 * DHW + kw + c0
                            rows = 64
                        nc.tensor.matmul(
                            dst,
                            wtile[0:rows, j * 64:(j + 1) * 64],
                            win[0:rows, off:off + csz],
                            start=(idx == first_ci[ci]),
                            stop=(idx == last_ci[ci]),
                        )
                    for ci in range(len(dps)):
                        nc.scalar.activation(
                            ot[ci * 64:(ci + 1) * 64, c0:c0 + csz],
                            pss[ci][ci * 64:(ci + 1) * 64, 0:csz],
                            mybir.ActivationFunctionType.Identity,
                            bias=btile[ci * 64:(ci + 1) * 64, :],
                        )
                nc.scalar.dma_start(out=y[grp, 0:nparts, :], in_=ot[0:nparts, :])
    nc.compile()
    return nc


def _prep_in_maps(x, s, style_weight, style_bias, weight, bias):
    style = s @ style_weight.T + style_bias                      # [N, Cin]
    wm = weight[None] * style[:, None, :, None, None, None]      # [N,Co,Ci,k,k,k]
    wm = wm * (1.0 / np.sqrt((wm * wm).sum(axis=(2, 3, 4, 5), keepdims=True) + EPS))
    wk = wm.transpose(0, 2, 3, 4, 5, 1)                          # [N,Ci,kd,kh,kw,Co]
    wfull = np.zeros((N, 128, NWBLK * COUT), np.float32)
    for j in range(9):
        kh, kw = divmod(j, 3)
        wfull[:, 0:64, j * 64:(j + 1) * 64] = wk[:, :, 0, kh, kw, :]
        wfull[:, 64:128, j * 64:(j + 1) * 64] = wk[:, :, 1, kh, kw, :]
    for kw in range(3):
        j = 9 + kw
        wfull[:, 0:64, j * 64:(j + 1) * 64] = wk[:, :, 2, 0, kw, :]
        wfull[:, 64:128, j * 64:(j + 1) * 64] = wk[:, :, 2, 1, kw, :]
    for kw in range(3):
        j = 12 + kw
        wfull[:, 0:64, j * 64:(j + 1) * 64] = wk[:, :, 2, 2, kw, :]
    wfull = np.ascontiguousarray(wfull.astype(NP_MM))
    bt = np.ascontiguousarray(
        np.tile(bias[:, None], (2, 1)), dtype=np.float32)        # [128,1]

    in_maps = []
    for core in range(NCORES):
        n, h = divmod(core, 2)
        d0 = h * HALF
        xsl = x[n, :, d0:d0 + P_IN].reshape(CIN, P_IN * PLANE)
        xsl = np.concatenate(
            [xsl, np.zeros((CIN, PAD_COLS), np.float32)], axis=1)
        in_maps.append({
            "xs": np.ascontiguousarray(xsl.astype(NP_MM)),
            "wt": wfull[n],
            "bt": bt,
        })
    return in_maps


def _gather(results):
    y = np.empty((N, COUT, DOUT, DOUT, DOUT), np.float32)
    for core in range(NCORES):
        n, h = divmod(core, 2)
        planes = results[core]["y"].reshape(
            NGROUPS * GROUP, COUT, DHW - 2, DHW)[:HALF]
        y[n, :, h * HALF:(h + 1) * HALF] = (
            planes[:, :, :, :DOUT].transpose(1, 0, 2, 3))
    return y


def kernel(x, s, style_weight, style_bias, weight, bias):
    global LAST_RESULTS
    x = np.asarray(x, np.float32)
    s = np.asarray(s, np.float32)
    style_weight = np.asarray(style_weight, np.float32)
    style_bias = np.asarray(style_bias, np.float32)
    weight = np.asarray(weight, np.float32)
    bias = np.asarray(bias, np.float32)

    if "nc" not in _CACHE:
        _CACHE["nc"] = _build_bass()
    in_maps = _prep_in_maps(x, s, style_weight, style_bias, weight, bias)
    res = None
    for attempt in range(3):
        try:
            res = run_bass_kernel_spmd(_CACHE["nc"], in_maps, list(range(NCORES)))
            break
        except Exception:
            if attempt == 2:
                raise
            time.sleep(30)  # transient device wedge; recovers on its own
    LAST_RESULTS = res
    return _gather(res.results)
